# revision 4
# baseline (speedup 1.0000x reference)
"""LIF spike-train kernel for Trainium2 (8 NeuronCores, SPMD data-parallel).

Recurrence per neuron over T=100 steps:
    mem_t   = DECAY * mem_{t-1} * (1 - spike_{t-1}) + x_t
    spike_t = (mem_t > THRESH)

Per step, two DVE ops + one ScalarE op (all bit-exact vs the reference):
    mem_t = (c_{t-1} mult DECAY) add x_t      -- one scalar_tensor_tensor
    c_t   = (mem_t is_le THRESH) mult mem_t   -- one scalar_tensor_tensor
    s_t   = Sign(mem_t - THRESH) -> u8 {0,1}  -- one ScalarE activation
The Sign u8 conversion maps {-1,0,+1} -> {0,0,1}, exactly the strict `>`.

Sharding: batch 128 -> 16 per core. Per core 65536 neurons laid out as
[128 partitions, 512 neurons]; input host-transposed to [128, 100, 512]
(partition, time, neuron) so every DMA run is contiguous per partition.
Output is uint8 [128, 100, 512] per core, converted to float32 on host.
"""

import sys

sys.path.insert(0, "/opt/trn_rl_repo")

import numpy as np

THRESH = 0.5
DECAY = 0.2
T = 100
P = 128
F = 512  # neurons per partition per core
N_CORES = 8
B_PER_CORE = 16  # 128 / 8
TC = 20  # time steps per DMA chunk
OUT_U8 = True
F_G = 144  # neurons per partition handled by GPSIMD (rest on DVE)
F_D = F - F_G


def _patch_tail_drain():
    """This container's walrus rejects >1 sync-wait on one CTRL instruction;
    spread the TileContext tail-drain waits across sync-engine NOPs."""
    from concourse import mybir, tile
    from concourse.vector_clock import ScopedClock

    if getattr(tile.TileContext, "_ant_drain_patched", False):
        return

    def _drain_and_barrier(self, tick_clock, wait_clock):
        nc = self.nc
        drain_inst = nc.sync.drain()
        wait_clock.add_sem_waits(
            drain_inst.ins, ScopedClock({None: tick_clock.global_clock})
        )
        si = drain_inst.ins.sync_info
        if si is not None and si.on_wait and len(si.on_wait) > 1:
            extra = list(si.on_wait)
            si.on_wait = []
            for i, w in enumerate(extra):
                nop = nc.sync.nop(hint=f"drain_split_{i}", nofuse=True)
                nsi = nop.ins.sync_info
                if nsi is None:
                    nop.ins.sync_info = mybir.SyncInfo(on_wait=[w], on_update=[])
                else:
                    nsi.on_wait = [w]
        nc.all_engine_barrier()
        popped = nc._tile_sem_poison_stack.pop()
        assert popped is self._sem_poison
        nc.clear_and_free_semaphores(list(self.sems.allocated().values()))
        nc.all_engine_barrier()

    tile.TileContext._drain_and_barrier = _drain_and_barrier
    tile.TileContext._ant_drain_patched = True


def _split_excess_waits(nc, max_waits=1):
    """Walrus in this container rejects instructions carrying more than a
    couple of sync waits; hoist excess waits onto same-engine NOPs placed
    immediately before the instruction (same per-engine program order)."""
    from concourse import mybir

    n_split = 0
    for fn in nc.m.functions:
        for bb in fn.blocks:
            out = []
            for ins in bb.instructions:
                si = getattr(ins, "sync_info", None)
                if si is not None and si.on_wait and len(si.on_wait) > max_waits:
                    waits = list(si.on_wait)
                    keep = waits[-max_waits:]
                    extra = waits[: -max_waits]
                    si.on_wait = keep
                    for j, w in enumerate(extra):
                        nop = mybir.InstNoOp(
                            name=f"{ins.name}-wsplit{j}",
                            engine=ins.engine,
                            bass_nofuse=True,
                            sync_info=mybir.SyncInfo(on_wait=[w], on_update=[]),
                        )
                        out.append(nop)
                        n_split += 1
                out.append(ins)
            bb.instructions = out
    return n_split


_nc_cache = None


def build_bass(reps=1):
    global _nc_cache
    if _nc_cache is not None and reps == 1:
        return _nc_cache
    from concourse import bass, mybir, tile

    _patch_tail_drain()

    out_dt = mybir.dt.uint8 if OUT_U8 else mybir.dt.float32
    f32 = mybir.dt.float32
    nc = bass.Bass()
    # Sign needs a const AP for its -THRESH bias; only 0.0/1.0 are built in.
    _bias_t = nc.alloc_sbuf_tensor("const-float32-negthresh", [128, 1], f32)
    nc.gpsimd.memset(_bias_t.ap(), -THRESH)
    nc.const_aps.aps[(f32, -THRESH)] = _bias_t.ap()
    nc.all_engine_barrier()
    x_ext = nc.declare_dram_parameter("x", [P, T, F], f32, isOutput=False)
    out_ext = nc.declare_dram_parameter("out", [P, T, F], out_dt, isOutput=True)

    with tile.TileContext(nc) as tc:
        with (
            tc.tile_pool(name="xin", bufs=3) as xin_pool,
            tc.tile_pool(name="sout", bufs=3) as sout_pool,
            tc.tile_pool(name="mem", bufs=4) as mem_pool,
            tc.tile_pool(name="carry", bufs=2) as carry_pool,
            tc.tile_pool(name="gtmp", bufs=3) as gtmp_pool,
        ):
          for _rep in range(reps):
            # mem_{-1} = 0; uniform loop from t=0
            mem = mem_pool.tile([P, F], f32)
            nc.vector.memset(mem[:, :F_D], 0.0)
            if F_G:
                nc.gpsimd.memset(mem[:, F_D:], 0.0)
            tin = None
            for ci in range(T // TC):
                tin_next = xin_pool.tile([P, TC, F], f32)
                nc.sync.dma_start(tin_next[:], x_ext[:, ci * TC : (ci + 1) * TC, :])
                tin = tin_next
                tout = sout_pool.tile([P, TC, F], out_dt)
                for tl in range(TC):
                    x_sl = tin[:, tl, :]
                    mem_next = mem_pool.tile([P, F], f32)
                    # DVE slice [:, :F_D]
                    c_d = carry_pool.tile([P, F_D], f32)
                    nc.vector.scalar_tensor_tensor(
                        c_d[:], mem[:, :F_D], THRESH, mem[:, :F_D],
                        mybir.AluOpType.is_le, mybir.AluOpType.mult,
                    )
                    nc.vector.scalar_tensor_tensor(
                        mem_next[:, :F_D], c_d[:], DECAY, x_sl[:, :F_D],
                        mybir.AluOpType.mult, mybir.AluOpType.add,
                    )
                    if F_G:
                        # GPSIMD slice [:, F_D:]: nsp = (mem<=0.5)*0.2;
                        # d = mem*nsp; mem' = d + x  (same fp32 rounding)
                        nsp_g = gtmp_pool.tile([P, F_G], f32)
                        nc.gpsimd.tensor_scalar(
                            nsp_g[:], mem[:, F_D:], THRESH, DECAY,
                            mybir.AluOpType.is_le, mybir.AluOpType.mult,
                        )
                        d_g = gtmp_pool.tile([P, F_G], f32)
                        nc.gpsimd.tensor_tensor(
                            d_g[:], mem[:, F_D:], nsp_g[:], mybir.AluOpType.mult
                        )
                        nc.gpsimd.tensor_tensor(
                            mem_next[:, F_D:], d_g[:], x_sl[:, F_D:],
                            mybir.AluOpType.add,
                        )
                    # spike: Sign(mem_next - THRESH) -> u8 {0,1}
                    nc.scalar.activation(
                        tout[:, tl, :], mem_next[:], mybir.ActivationFunctionType.Sign,
                        bias=-THRESH, scale=1.0,
                    )
                    mem = mem_next
                nc.sync.dma_start(out_ext[:, ci * TC : (ci + 1) * TC, :], tout[:])

    _split_excess_waits(nc, max_waits=1)
    if reps == 1:
        _nc_cache = nc
    return nc


def _prep_core_input(xc):
    # xc: [16, 4096, 100] fp32 -> [128, 100, 512] (partition, time, neuron)
    return np.ascontiguousarray(xc.reshape(P, F, T).transpose(0, 2, 1))


def _unprep_core_output(oc):
    # oc: [128, 100, 512] -> [16, 4096, 100] fp32
    return oc.transpose(0, 2, 1).reshape(B_PER_CORE, 4096, T).astype(np.float32)


def kernel(x, _trace=False, _trace_kwargs=None):
    from concourse.bass_utils import run_bass_kernel_spmd

    nc = build_bass()
    xs = x.reshape(N_CORES, B_PER_CORE, 4096, T)
    in_maps = [{"x": _prep_core_input(xs[c])} for c in range(N_CORES)]
    kw = {}
    if _trace:
        kw["trace"] = True
        kw.update(_trace_kwargs or {})
    res = run_bass_kernel_spmd(nc, in_maps, list(range(N_CORES)), **kw)
    out = np.concatenate(
        [_unprep_core_output(res.results[c]["out"]) for c in range(N_CORES)], axis=0
    )
    if _trace:
        return out, res
    return out


# revision 5
# speedup vs baseline: 3.4251x; 3.4251x over previous
"""LIF spike-train kernel for Trainium2 (8 NeuronCores, SPMD data-parallel).

Recurrence per neuron over T=100 steps:
    mem_t   = DECAY * mem_{t-1} * (1 - spike_{t-1}) + x_t
    spike_t = (mem_t > THRESH)

Per step, two DVE ops + one ScalarE op (all bit-exact vs the reference):
    mem_t = (c_{t-1} mult DECAY) add x_t      -- one scalar_tensor_tensor
    c_t   = (mem_t is_le THRESH) mult mem_t   -- one scalar_tensor_tensor
    s_t   = Sign(mem_t - THRESH) -> u8 {0,1}  -- one ScalarE activation
The Sign u8 conversion maps {-1,0,+1} -> {0,0,1}, exactly the strict `>`.

Sharding: batch 128 -> 16 per core. Per core 65536 neurons laid out as
[128 partitions, 512 neurons]; input host-transposed to [128, 100, 512]
(partition, time, neuron) so every DMA run is contiguous per partition.
Output is uint8 [128, 100, 512] per core, converted to float32 on host.
"""

import sys

sys.path.insert(0, "/opt/trn_rl_repo")

import numpy as np

THRESH = 0.5
DECAY = 0.2
T = 100
P = 128
F = 512  # neurons per partition per core
N_CORES = 8
B_PER_CORE = 16  # 128 / 8
TC = 25  # time steps per DMA chunk
OUT_U8 = True
F_G = 0  # neurons per partition handled by GPSIMD (rest on DVE); 0 = DVE only
F_D = F - F_G


def _patch_tail_drain():
    """This container's walrus rejects >1 sync-wait on one CTRL instruction;
    spread the TileContext tail-drain waits across sync-engine NOPs."""
    from concourse import mybir, tile
    from concourse.vector_clock import ScopedClock

    if getattr(tile.TileContext, "_ant_drain_patched", False):
        return

    def _drain_and_barrier(self, tick_clock, wait_clock):
        nc = self.nc
        drain_inst = nc.sync.drain()
        wait_clock.add_sem_waits(
            drain_inst.ins, ScopedClock({None: tick_clock.global_clock})
        )
        si = drain_inst.ins.sync_info
        if si is not None and si.on_wait and len(si.on_wait) > 1:
            extra = list(si.on_wait)
            si.on_wait = []
            for i, w in enumerate(extra):
                nop = nc.sync.nop(hint=f"drain_split_{i}", nofuse=True)
                nsi = nop.ins.sync_info
                if nsi is None:
                    nop.ins.sync_info = mybir.SyncInfo(on_wait=[w], on_update=[])
                else:
                    nsi.on_wait = [w]
        nc.all_engine_barrier()
        popped = nc._tile_sem_poison_stack.pop()
        assert popped is self._sem_poison
        nc.clear_and_free_semaphores(list(self.sems.allocated().values()))
        nc.all_engine_barrier()

    tile.TileContext._drain_and_barrier = _drain_and_barrier
    tile.TileContext._ant_drain_patched = True


def _split_excess_waits(nc, max_waits=1):
    """Walrus in this container rejects instructions carrying more than a
    couple of sync waits; hoist excess waits onto same-engine NOPs placed
    immediately before the instruction (same per-engine program order)."""
    from concourse import mybir

    n_split = 0
    for fn in nc.m.functions:
        for bb in fn.blocks:
            out = []
            for ins in bb.instructions:
                si = getattr(ins, "sync_info", None)
                if si is not None and si.on_wait and len(si.on_wait) > max_waits:
                    waits = list(si.on_wait)
                    keep = waits[-max_waits:]
                    extra = waits[: -max_waits]
                    si.on_wait = keep
                    for j, w in enumerate(extra):
                        nop = mybir.InstNoOp(
                            name=f"{ins.name}-wsplit{j}",
                            engine=ins.engine,
                            bass_nofuse=True,
                            sync_info=mybir.SyncInfo(on_wait=[w], on_update=[]),
                        )
                        out.append(nop)
                        n_split += 1
                out.append(ins)
            bb.instructions = out
    return n_split


_nc_cache = None


def build_bass(reps=1):
    global _nc_cache
    if _nc_cache is not None and reps == 1:
        return _nc_cache
    from concourse import bass, mybir, tile

    _patch_tail_drain()

    out_dt = mybir.dt.uint8 if OUT_U8 else mybir.dt.float32
    f32 = mybir.dt.float32
    nc = bass.Bass()
    # Sign needs a const AP for its -THRESH bias; only 0.0/1.0 are built in.
    _bias_t = nc.alloc_sbuf_tensor("const-float32-negthresh", [128, 1], f32)
    nc.gpsimd.memset(_bias_t.ap(), -THRESH)
    nc.const_aps.aps[(f32, -THRESH)] = _bias_t.ap()
    nc.all_engine_barrier()
    x_ext = nc.declare_dram_parameter("x", [P, T, F], f32, isOutput=False)
    out_ext = nc.declare_dram_parameter("out", [P, T, F], out_dt, isOutput=True)

    with tile.TileContext(nc) as tc:
        with (
            tc.tile_pool(name="xin", bufs=2) as xin_pool,
            tc.tile_pool(name="sout", bufs=2) as sout_pool,
            tc.tile_pool(name="mem", bufs=4) as mem_pool,
            tc.tile_pool(name="carry", bufs=2) as carry_pool,
            tc.tile_pool(name="gtmp", bufs=3) as gtmp_pool,
        ):
          for _rep in range(reps):
            # mem_{-1} = 0; uniform loop from t=0
            mem = mem_pool.tile([P, F], f32)
            nc.vector.memset(mem[:, :F_D], 0.0)
            if F_G:
                nc.gpsimd.memset(mem[:, F_D:], 0.0)
            tin = None
            for ci in range(T // TC):
                tin_next = xin_pool.tile([P, TC, F], f32)
                nc.sync.dma_start(tin_next[:], x_ext[:, ci * TC : (ci + 1) * TC, :])
                tin = tin_next
                tout = sout_pool.tile([P, TC, F], out_dt)
                for tl in range(TC):
                    x_sl = tin[:, tl, :]
                    mem_next = mem_pool.tile([P, F], f32)
                    # DVE slice [:, :F_D]
                    c_d = carry_pool.tile([P, F_D], f32)
                    nc.vector.scalar_tensor_tensor(
                        c_d[:], mem[:, :F_D], THRESH, mem[:, :F_D],
                        mybir.AluOpType.is_le, mybir.AluOpType.mult,
                    )
                    nc.vector.scalar_tensor_tensor(
                        mem_next[:, :F_D], c_d[:], DECAY, x_sl[:, :F_D],
                        mybir.AluOpType.mult, mybir.AluOpType.add,
                    )
                    if F_G:
                        # GPSIMD slice [:, F_D:]: nsp = (mem<=0.5)*0.2;
                        # d = mem*nsp; mem' = d + x  (same fp32 rounding)
                        nsp_g = gtmp_pool.tile([P, F_G], f32)
                        nc.gpsimd.tensor_scalar(
                            nsp_g[:], mem[:, F_D:], THRESH, DECAY,
                            mybir.AluOpType.is_le, mybir.AluOpType.mult,
                        )
                        d_g = gtmp_pool.tile([P, F_G], f32)
                        nc.gpsimd.tensor_tensor(
                            d_g[:], mem[:, F_D:], nsp_g[:], mybir.AluOpType.mult
                        )
                        nc.gpsimd.tensor_tensor(
                            mem_next[:, F_D:], d_g[:], x_sl[:, F_D:],
                            mybir.AluOpType.add,
                        )
                    # spike: Sign(mem_next - THRESH) -> u8 {0,1}
                    nc.scalar.activation(
                        tout[:, tl, :], mem_next[:], mybir.ActivationFunctionType.Sign,
                        bias=-THRESH, scale=1.0,
                    )
                    mem = mem_next
                nc.scalar.dma_start(out_ext[:, ci * TC : (ci + 1) * TC, :], tout[:])

    _split_excess_waits(nc, max_waits=1)
    if reps == 1:
        _nc_cache = nc
    return nc


def _prep_core_input(xc):
    # xc: [16, 4096, 100] fp32 -> [128, 100, 512] (partition, time, neuron)
    return np.ascontiguousarray(xc.reshape(P, F, T).transpose(0, 2, 1))


def _unprep_core_output(oc):
    # oc: [128, 100, 512] -> [16, 4096, 100] fp32
    return oc.transpose(0, 2, 1).reshape(B_PER_CORE, 4096, T).astype(np.float32)


def kernel(x, _trace=False, _trace_kwargs=None):
    from concourse.bass_utils import run_bass_kernel_spmd

    nc = build_bass()
    xs = x.reshape(N_CORES, B_PER_CORE, 4096, T)
    in_maps = [{"x": _prep_core_input(xs[c])} for c in range(N_CORES)]
    kw = {}
    if _trace:
        kw["trace"] = True
        kw.update(_trace_kwargs or {})
    res = run_bass_kernel_spmd(nc, in_maps, list(range(N_CORES)), **kw)
    out = np.concatenate(
        [_unprep_core_output(res.results[c]["out"]) for c in range(N_CORES)], axis=0
    )
    if _trace:
        return out, res
    return out
